# revision 3
# baseline (speedup 1.0000x reference)
"""Multi-head attention (QKV proj + softmax(QK^T)V) on 8 TRN2 NeuronCores.

Sharding: 8 cores = 4 batches x 2 head-groups (6 heads each). Pure data
parallel - no collectives. Host pre-transposes shards so every on-device
matmul streams with zero on-chip transposes:
  per core: qT,kT,vT [768,2048] bf16, WqT,WkT,WvT [768,384] bf16,
            bqT,bkT [128,3] f32, bv_rep [128,384] f32  ->  outT [384,2048] f32

Per-core pipeline (all layouts transposed, d-on-partitions):
  wqT = WqT.T @ qT + bq      [384,2048]  (pair p -> partitions: head 2p = 0:64, 2p+1 = 64:128)
  wv  = vT.T @ WvT + bv      [2048,384]  (stored per seq-tile with a ones column per head)
  per head: S^T = wkT.T @ wqT  -> exp on ScalarE (no max subtraction; scores <~70, fp32-safe)
            [out.T; rowsum] = [wv | 1].T @ P^T   (softmax denominator rides the AV matmul)
  normalize: recip on VectorE, broadcast across partitions via K=1 fp32 matmul, multiply.
"""

import sys

if "/opt/trn_rl_repo" not in sys.path:
    sys.path.insert(0, "/opt/trn_rl_repo")

import numpy as np

_BS, _SEQ, _DM = 4, 2048, 768
_NH, _DH = 12, 64
_GSZ = _DM // 2  # 384 dims per head-group
_NCORES = 8

_compiled = None


def _build():
    import concourse.bass as bass  # noqa: F401
    import concourse.mybir as mybir
    import concourse.tile as tile
    from concourse import bacc

    f32 = mybir.dt.float32
    bf16 = mybir.dt.bfloat16
    AF = mybir.ActivationFunctionType

    nc = bacc.Bacc("TRN2", target_bir_lowering=False, debug=False)

    qT = nc.dram_tensor("qT", [_DM, _SEQ], bf16, kind="ExternalInput")
    kT = nc.dram_tensor("kT", [_DM, _SEQ], bf16, kind="ExternalInput")
    vT = nc.dram_tensor("vT", [_DM, _SEQ], bf16, kind="ExternalInput")
    WqT = nc.dram_tensor("WqT", [_DM, _GSZ], bf16, kind="ExternalInput")
    WkT = nc.dram_tensor("WkT", [_DM, _GSZ], bf16, kind="ExternalInput")
    WvT = nc.dram_tensor("WvT", [_DM, _GSZ], bf16, kind="ExternalInput")
    bqT = nc.dram_tensor("bqT", [128, 3], f32, kind="ExternalInput")
    bkT = nc.dram_tensor("bkT", [128, 3], f32, kind="ExternalInput")
    bv_rep = nc.dram_tensor("bv_rep", [128, _GSZ], f32, kind="ExternalInput")
    outT = nc.dram_tensor("outT", [_GSZ, _SEQ], f32, kind="ExternalOutput")

    KT = _DM // 128  # 6 contraction tiles for projections
    ST = _SEQ // 128  # 16 seq tiles (key positions)
    QC = _SEQ // 512  # 4 query chunks
    NP = _GSZ // 128  # 3 head pairs

    with tile.TileContext(nc) as tc:
        with (
            tc.tile_pool(name="persist", bufs=1) as persist,
            tc.tile_pool(name="qkv", bufs=1) as qkv_pool,
            tc.tile_pool(name="w", bufs=1) as w_pool,
        ):
            # ---- persistent SBUF tensors ----
            wqT_sb = [persist.tile([128, _SEQ], bf16, tag=f"wqT{p}", name=f"wqT{p}") for p in range(NP)]
            wkT_sb = [persist.tile([128, _SEQ], bf16, tag=f"wkT{p}", name=f"wkT{p}") for p in range(NP)]
            # per seq-tile, per head: [64 wv dims | ones | pad]
            wv_sb = persist.tile([128, ST, 6, 66], bf16, tag="wv")
            ones_sb = persist.tile([1, 64], f32, tag="ones")
            nc.vector.memset(ones_sb[:, :], 1.0)
            for st in range(ST):
                nc.vector.memset(wv_sb[:, st, :, 64:65], 1.0)

            # ---- load inputs ----
            qT_sb = [qkv_pool.tile([128, _SEQ], bf16, tag=f"qT{t}", name=f"qTs{t}") for t in range(KT)]
            kT_sb = [qkv_pool.tile([128, _SEQ], bf16, tag=f"kT{t}", name=f"kTs{t}") for t in range(KT)]
            vT_sb = [qkv_pool.tile([128, _SEQ], bf16, tag=f"vT{t}", name=f"vTs{t}") for t in range(KT)]
            WqT_sb = [w_pool.tile([128, _GSZ], bf16, tag=f"Wq{t}", name=f"Wqs{t}") for t in range(KT)]
            WkT_sb = [w_pool.tile([128, _GSZ], bf16, tag=f"Wk{t}", name=f"Wks{t}") for t in range(KT)]
            WvT_sb = [w_pool.tile([128, _GSZ], bf16, tag=f"Wv{t}", name=f"Wvs{t}") for t in range(KT)]
            bqT_sb = persist.tile([128, 3], f32, tag="bqT")
            bkT_sb = persist.tile([128, 3], f32, tag="bkT")
            bv_sb = persist.tile([128, _GSZ], f32, tag="bv")
            for t in range(KT):
                sl = slice(t * 128, (t + 1) * 128)
                nc.sync.dma_start(WqT_sb[t][:, :], WqT[sl, :])
                nc.sync.dma_start(WkT_sb[t][:, :], WkT[sl, :])
                nc.sync.dma_start(WvT_sb[t][:, :], WvT[sl, :])
                nc.sync.dma_start(qT_sb[t][:, :], qT[sl, :])
                nc.sync.dma_start(kT_sb[t][:, :], kT[sl, :])
                nc.sync.dma_start(vT_sb[t][:, :], vT[sl, :])
            nc.sync.dma_start(bqT_sb[:, :], bqT[:, :])
            nc.sync.dma_start(bkT_sb[:, :], bkT[:, :])
            nc.sync.dma_start(bv_sb[:, :], bv_rep[:, :])

            # ---- phase P: projections ----
            with tc.tile_pool(name="psum_p", bufs=2, space="PSUM") as psum_p:
                for m in range(NP):
                    msl = slice(m * 128, (m + 1) * 128)
                    for nch in range(QC):
                        nsl = slice(nch * 512, (nch + 1) * 512)
                        psq = psum_p.tile([128, 512], f32, tag="projq")
                        psk = psum_p.tile([128, 512], f32, tag="projk")
                        for t in range(KT):
                            nc.tensor.matmul(
                                psq[:, :], WqT_sb[t][:, msl], qT_sb[t][:, nsl],
                                start=(t == 0), stop=(t == KT - 1),
                            )
                        for t in range(KT):
                            nc.tensor.matmul(
                                psk[:, :], WkT_sb[t][:, msl], kT_sb[t][:, nsl],
                                start=(t == 0), stop=(t == KT - 1),
                            )
                        nc.vector.tensor_scalar_add(
                            wqT_sb[m][:, nsl], psq[:, :], bqT_sb[:, m:m + 1]
                        )
                        nc.vector.tensor_scalar_add(
                            wkT_sb[m][:, nsl], psk[:, :], bkT_sb[:, m:m + 1]
                        )
                for st in range(ST):
                    ssl = slice(st * 128, (st + 1) * 128)
                    psv = psum_p.tile([128, _GSZ], f32, tag="projv")
                    for t in range(KT):
                        nc.tensor.matmul(
                            psv[:, :], vT_sb[t][:, ssl], WvT_sb[t][:, :],
                            start=(t == 0), stop=(t == KT - 1),
                        )
                    for h in range(6):
                        hsl = slice(h * 64, (h + 1) * 64)
                        nc.vector.tensor_add(
                            wv_sb[:, st, h, 0:64], psv[:, hsl], bv_sb[:, hsl]
                        )

            # ---- phase A: attention ----
            with (
                tc.tile_pool(name="psum_a", bufs=2, space="PSUM") as psum_a,
                tc.tile_pool(name="psum_av", bufs=1, space="PSUM") as psum_av,
                tc.tile_pool(name="att", bufs=3) as att_pool,
            ):
                for p in range(NP):
                    hA, hB = 2 * p, 2 * p + 1
                    for qch in range(QC):
                        qsl = slice(qch * 512, (qch + 1) * 512)
                        avA = psum_av.tile([128, 512], f32, tag="avA")
                        avB = psum_av.tile([128, 512], f32, tag="avB")
                        s_tiles = []
                        p_tiles = []
                        for kt in range(ST):
                            ksl = slice(kt * 128, (kt + 1) * 128)
                            s_AB = psum_a.tile([128, 1024], f32, tag="s")
                            nc.tensor.matmul(
                                s_AB[:, 0:512],
                                wkT_sb[p][0:64, ksl], wqT_sb[p][0:64, qsl],
                                start=True, stop=True,
                            )
                            nc.tensor.matmul(
                                s_AB[:, 512:1024],
                                wkT_sb[p][64:128, ksl], wqT_sb[p][64:128, qsl],
                                start=True, stop=True,
                            )
                            p_AB = att_pool.tile([128, 1024], bf16, tag="p")
                            nc.scalar.activation(p_AB[:, :], s_AB[:, :], AF.Exp)
                            s_tiles.append(s_AB)
                            p_tiles.append(p_AB)
                            # software pipeline: AV for kt-1 after S/exp of kt
                            if kt >= 1:
                                pv = p_tiles[kt - 1]
                                nc.tensor.matmul(
                                    avA[0:65, :], wv_sb[:, kt - 1, hA, 0:65],
                                    pv[:, 0:512],
                                    start=(kt - 1 == 0), stop=False,
                                )
                                nc.tensor.matmul(
                                    avB[0:65, :], wv_sb[:, kt - 1, hB, 0:65],
                                    pv[:, 512:1024],
                                    start=(kt - 1 == 0), stop=False,
                                )
                        pv = p_tiles[ST - 1]
                        nc.tensor.matmul(
                            avA[0:65, :], wv_sb[:, ST - 1, hA, 0:65], pv[:, 0:512],
                            start=False, stop=True,
                        )
                        nc.tensor.matmul(
                            avB[0:65, :], wv_sb[:, ST - 1, hB, 0:65], pv[:, 512:1024],
                            start=False, stop=True,
                        )
                        # normalize: out = av[0:64] * (1/av[64])
                        recip = att_pool.tile([1, 1024], f32, tag="recip")
                        nc.vector.reciprocal(recip[0:1, 0:512], avA[64:65, :])
                        nc.vector.reciprocal(recip[0:1, 512:1024], avB[64:65, :])
                        bc = psum_a.tile([128, 1024], f32, tag="s")
                        nc.tensor.matmul(
                            bc[0:64, 0:512], ones_sb[0:1, :], recip[0:1, 0:512],
                            start=True, stop=True,
                        )
                        nc.tensor.matmul(
                            bc[0:64, 512:1024], ones_sb[0:1, :], recip[0:1, 512:1024],
                            start=True, stop=True,
                        )
                        bc_sb = att_pool.tile([64, 1024], f32, tag="bc_sb")
                        nc.vector.tensor_copy(bc_sb[0:64, :], bc[0:64, :])
                        o_sb = att_pool.tile([64, 1024], f32, tag="o")
                        nc.vector.tensor_mul(
                            o_sb[0:64, 0:512], avA[0:64, :], bc_sb[0:64, 0:512]
                        )
                        nc.vector.tensor_mul(
                            o_sb[0:64, 512:1024], avB[0:64, :], bc_sb[0:64, 512:1024]
                        )
                        nc.sync.dma_start(
                            outT[hA * 64:hA * 64 + 64, qsl], o_sb[0:64, 0:512]
                        )
                        nc.sync.dma_start(
                            outT[hB * 64:hB * 64 + 64, qsl], o_sb[0:64, 512:1024]
                        )

    nc.compile()
    return nc


def _get_compiled():
    global _compiled
    if _compiled is None:
        _compiled = _build()
    return _compiled


def make_in_maps(q, k, v, Wq, bq, Wk, bk, Wv, bv):
    import ml_dtypes

    bf16 = ml_dtypes.bfloat16
    in_maps = []
    for c in range(_NCORES):
        b, g = c // 2, c % 2
        gsl = slice(g * _GSZ, (g + 1) * _GSZ)
        in_maps.append({
            "qT": np.ascontiguousarray(np.asarray(q)[b].T).astype(bf16),
            "kT": np.ascontiguousarray(np.asarray(k)[b].T).astype(bf16),
            "vT": np.ascontiguousarray(np.asarray(v)[b].T).astype(bf16),
            "WqT": np.ascontiguousarray(np.asarray(Wq)[gsl, :].T).astype(bf16),
            "WkT": np.ascontiguousarray(np.asarray(Wk)[gsl, :].T).astype(bf16),
            "WvT": np.ascontiguousarray(np.asarray(Wv)[gsl, :].T).astype(bf16),
            "bqT": np.ascontiguousarray(
                np.asarray(bq)[gsl].reshape(3, 128).T
            ).astype(np.float32),
            "bkT": np.ascontiguousarray(
                np.asarray(bk)[gsl].reshape(3, 128).T
            ).astype(np.float32),
            "bv_rep": np.tile(
                np.asarray(bv)[gsl][None, :], (128, 1)
            ).astype(np.float32),
        })
    return in_maps


def assemble_out(results):
    out = np.zeros((_BS, _SEQ, _DM), np.float32)
    for c in range(_NCORES):
        b, g = c // 2, c % 2
        out[b, :, g * _GSZ:(g + 1) * _GSZ] = np.asarray(
            results[c]["outT"], np.float32
        ).T
    return out


def kernel(q, k, v, Wq, bq, Wk, bk, Wv, bv):
    from concourse.bass_utils import run_bass_kernel_spmd

    nc = _get_compiled()
    in_maps = make_in_maps(q, k, v, Wq, bq, Wk, bk, Wv, bv)
    res = run_bass_kernel_spmd(nc, in_maps, core_ids=list(range(_NCORES)))
    return assemble_out(res.results)


# revision 9
# speedup vs baseline: 1.1450x; 1.1450x over previous
"""Multi-head attention (QKV proj + softmax(QK^T)V) on 8 TRN2 NeuronCores.

Sharding: 8 cores = 4 batches x 2 head-groups (6 heads each). Pure data
parallel - no collectives. Host pre-transposes shards so every on-device
matmul streams with zero on-chip transposes:
  per core: qT,kT,vT [768,2048] bf16, WqT,WkT,WvT [768,384] bf16,
            bqT,bkT [128,3] f32, bv_rep [128,384] f32  ->  outT [384,2048] f32

Per-core pipeline (all layouts transposed, d-on-partitions):
  wqT = WqT.T @ qT + bq      [384,2048]  (pair p -> partitions: head 2p = 0:64, 2p+1 = 64:128)
  wv  = vT.T @ WvT + bv      [2048,384]  (stored per seq-tile with a ones column per head)
  per head: S^T = wkT.T @ wqT  -> exp on ScalarE (no max subtraction; scores <~70, fp32-safe)
            [out.T; rowsum] = [wv | 1].T @ P^T   (softmax denominator rides the AV matmul)
  normalize: recip on VectorE, broadcast across partitions via K=1 fp32 matmul, multiply.
"""

import sys

if "/opt/trn_rl_repo" not in sys.path:
    sys.path.insert(0, "/opt/trn_rl_repo")

import os

import numpy as np

_TILEPOS = os.environ.get("K_TILEPOS", "1") == "1"
_PBCAST = os.environ.get("K_PBCAST", "1") == "1"
_DIVIDE = os.environ.get("K_DIVIDE", "0") == "1"  # DVE has no divide ALU op (s3s3d3_tt_valid_op)
_RECIP_FAST = os.environ.get("K_RECIP_FAST", "0") == "1"  # approx_fast gives wrong results on HW

_BS, _SEQ, _DM = 4, 2048, 768
_NH, _DH = 12, 64
_GSZ = _DM // 2  # 384 dims per head-group
_NCORES = 8

_compiled = None


def _build():
    import concourse.bass as bass  # noqa: F401
    import concourse.mybir as mybir
    import concourse.tile as tile
    from concourse import bacc

    f32 = mybir.dt.float32
    bf16 = mybir.dt.bfloat16
    AF = mybir.ActivationFunctionType

    nc = bacc.Bacc("TRN2", target_bir_lowering=False, debug=False)

    qT = nc.dram_tensor("qT", [_DM, _SEQ], bf16, kind="ExternalInput")
    kT = nc.dram_tensor("kT", [_DM, _SEQ], bf16, kind="ExternalInput")
    vT = nc.dram_tensor("vT", [_DM, _SEQ], bf16, kind="ExternalInput")
    WqT = nc.dram_tensor("WqT", [_DM, _GSZ], bf16, kind="ExternalInput")
    WkT = nc.dram_tensor("WkT", [_DM, _GSZ], bf16, kind="ExternalInput")
    WvT = nc.dram_tensor("WvT", [_DM, _GSZ], bf16, kind="ExternalInput")
    bqT = nc.dram_tensor("bqT", [128, 3], f32, kind="ExternalInput")
    bkT = nc.dram_tensor("bkT", [128, 3], f32, kind="ExternalInput")
    bv_rep = nc.dram_tensor("bv_rep", [128, _GSZ], f32, kind="ExternalInput")
    outT = nc.dram_tensor("outT", [_GSZ, _SEQ], f32, kind="ExternalOutput")

    KT = _DM // 128  # 6 contraction tiles for projections
    ST = _SEQ // 128  # 16 seq tiles (key positions)
    QC = _SEQ // 512  # 4 query chunks
    NP = _GSZ // 128  # 3 head pairs

    with tile.TileContext(nc) as tc:
        with (
            tc.tile_pool(name="persist", bufs=1) as persist,
            tc.tile_pool(name="qkv", bufs=1) as qkv_pool,
            tc.tile_pool(name="w", bufs=1) as w_pool,
            tc.tile_pool(name="psum", bufs=2, space="PSUM") as psum,
            tc.tile_pool(name="att", bufs=3) as att_pool,
        ):
            # ---- persistent SBUF tensors ----
            wqT_sb = [persist.tile([128, _SEQ], bf16, tag=f"wqT{p}", name=f"wqT{p}") for p in range(NP)]
            wkT_sb = [persist.tile([128, _SEQ], bf16, tag=f"wkT{p}", name=f"wkT{p}") for p in range(NP)]
            # per seq-tile, per head: [64 wv dims | ones | pad]
            wv_sb = persist.tile([128, ST, 6, 66], bf16, tag="wv")
            for st in range(ST):
                nc.vector.memset(wv_sb[:, st, :, 64:65], 1.0)

            # ---- load inputs ----
            qT_sb = [qkv_pool.tile([128, _SEQ], bf16, tag=f"qT{t}", name=f"qTs{t}") for t in range(KT)]
            kT_sb = [qkv_pool.tile([128, _SEQ], bf16, tag=f"kT{t}", name=f"kTs{t}") for t in range(KT)]
            vT_sb = [qkv_pool.tile([128, _SEQ], bf16, tag=f"vT{t}", name=f"vTs{t}") for t in range(KT)]
            WqT_sb = [w_pool.tile([128, _GSZ], bf16, tag=f"Wq{t}", name=f"Wqs{t}") for t in range(KT)]
            WkT_sb = [w_pool.tile([128, _GSZ], bf16, tag=f"Wk{t}", name=f"Wks{t}") for t in range(KT)]
            WvT_sb = [w_pool.tile([128, _GSZ], bf16, tag=f"Wv{t}", name=f"Wvs{t}") for t in range(KT)]
            bqT_sb = persist.tile([128, 3], f32, tag="bqT")
            bkT_sb = persist.tile([128, 3], f32, tag="bkT")
            bv_sb = persist.tile([128, _GSZ], f32, tag="bv")
            for t in range(KT):
                sl = slice(t * 128, (t + 1) * 128)
                nc.sync.dma_start(WvT_sb[t][:, :], WvT[sl, :])
                nc.sync.dma_start(vT_sb[t][:, :], vT[sl, :])
                nc.sync.dma_start(WqT_sb[t][:, :], WqT[sl, :])
                nc.sync.dma_start(WkT_sb[t][:, :], WkT[sl, :])
                nc.sync.dma_start(qT_sb[t][:, :], qT[sl, :])
                nc.sync.dma_start(kT_sb[t][:, :], kT[sl, :])
            nc.sync.dma_start(bqT_sb[:, :], bqT[:, :])
            nc.sync.dma_start(bkT_sb[:, :], bkT[:, :])
            nc.sync.dma_start(bv_sb[:, :], bv_rep[:, :])

            # ---- projection unit emitters (psum shares tag "s" with attention) ----
            def emit_v_unit(st):
                ssl = slice(st * 128, (st + 1) * 128)
                psv = psum.tile([128, _GSZ], f32, tag="s", name="psv",
                                padded_shape=[128, 1024])
                for t in range(KT):
                    nc.tensor.matmul(
                        psv[:, :], vT_sb[t][:, ssl], WvT_sb[t][:, :],
                        start=(t == 0), stop=(t == KT - 1),
                    )
                for h in range(6):
                    hsl = slice(h * 64, (h + 1) * 64)
                    nc.vector.tensor_add(
                        wv_sb[:, st, h, 0:64], psv[:, hsl], bv_sb[:, hsl]
                    )

            def emit_qk_unit(which, m, nch):
                msl = slice(m * 128, (m + 1) * 128)
                nsl = slice(nch * 512, (nch + 1) * 512)
                ps = psum.tile([128, 512], f32, tag="s", name="psqk",
                               padded_shape=[128, 1024])
                W_sb, x_sb, dst, b_sb = (
                    (WqT_sb, qT_sb, wqT_sb, bqT_sb) if which == "q"
                    else (WkT_sb, kT_sb, wkT_sb, bkT_sb)
                )
                for t in range(KT):
                    nc.tensor.matmul(
                        ps[:, :], W_sb[t][:, msl], x_sb[t][:, nsl],
                        start=(t == 0), stop=(t == KT - 1),
                    )
                nc.vector.tensor_scalar_add(dst[m][:, nsl], ps[:, :], b_sb[:, m:m + 1])

            # ---- phase P head: V projections + pair-0 Q/K ----
            for st in range(ST):
                emit_v_unit(st)
            for nch in range(QC):
                emit_qk_unit("q", 0, nch)
                emit_qk_unit("k", 0, nch)

            # ---- attention (remaining projections interleaved at tails) ----
            for p in range(NP):
                hA, hB = 2 * p, 2 * p + 1
                for qch in range(QC):
                    qsl = slice(qch * 512, (qch + 1) * 512)
                    avA = psum.tile([128, 512], f32, tag="avA", name="avA")
                    avB = psum.tile([128, 512], f32, tag="avB", name="avB")
                    p_tiles = []
                    for kt in range(ST):
                        ksl = slice(kt * 128, (kt + 1) * 128)
                        s_AB = psum.tile([128, 1024], f32, tag="s", name="sAB")
                        nc.tensor.matmul(
                            s_AB[:, 0:512],
                            wkT_sb[p][0:64, ksl], wqT_sb[p][0:64, qsl],
                            start=True, stop=True,
                            tile_position=(0, 0) if _TILEPOS else None,
                        )
                        nc.tensor.matmul(
                            s_AB[:, 512:1024],
                            wkT_sb[p][64:128, ksl], wqT_sb[p][64:128, qsl],
                            start=True, stop=True,
                            tile_position=(64, 0) if _TILEPOS else None,
                        )
                        p_AB = att_pool.tile([128, 1024], bf16, tag="p", name="pAB")
                        nc.scalar.activation(p_AB[:, :], s_AB[:, :], AF.Exp)
                        p_tiles.append(p_AB)
                        if kt >= 1:
                            pv = p_tiles[kt - 1]
                            nc.tensor.matmul(
                                avA[0:65, :], wv_sb[:, kt - 1, hA, 0:65],
                                pv[:, 0:512],
                                start=(kt - 1 == 0), stop=False,
                            )
                            nc.tensor.matmul(
                                avB[0:65, :], wv_sb[:, kt - 1, hB, 0:65],
                                pv[:, 512:1024],
                                start=(kt - 1 == 0), stop=False,
                            )
                    pv = p_tiles[ST - 1]
                    nc.tensor.matmul(
                        avA[0:65, :], wv_sb[:, ST - 1, hA, 0:65], pv[:, 0:512],
                        start=False, stop=True,
                    )
                    nc.tensor.matmul(
                        avB[0:65, :], wv_sb[:, ST - 1, hB, 0:65], pv[:, 512:1024],
                        start=False, stop=True,
                    )
                    # normalize: out = av[0:64] / av[64] (sums -> sbuf ->
                    # partition-broadcast on GpSimd -> DVE divide)
                    if _DIVIDE:
                        sums_sb = att_pool.tile([1, 1024], f32, tag="sums", name="sums")
                        nc.vector.tensor_copy(sums_sb[0:1, 0:512], avA[64:65, :])
                        nc.vector.tensor_copy(sums_sb[0:1, 512:1024], avB[64:65, :])
                        bc_sb = att_pool.tile([64, 1024], f32, tag="bc_sb", name="bc_sb")
                        nc.gpsimd.partition_broadcast(bc_sb[0:64, :], sums_sb[0:1, :])
                        o_sb = att_pool.tile([64, 1024], f32, tag="o", name="o_sb")
                        nc.vector.tensor_tensor(
                            o_sb[0:64, 0:512], avA[0:64, :], bc_sb[0:64, 0:512],
                            op=mybir.AluOpType.divide,
                        )
                        nc.vector.tensor_tensor(
                            o_sb[0:64, 512:1024], avB[0:64, :], bc_sb[0:64, 512:1024],
                            op=mybir.AluOpType.divide,
                        )
                    else:
                        recip = att_pool.tile([1, 1024], f32, tag="sums", name="recip")
                        nc.vector.reciprocal(recip[0:1, 0:512], avA[64:65, :])
                        nc.vector.reciprocal(recip[0:1, 512:1024], avB[64:65, :])
                        bc_sb = att_pool.tile([64, 1024], f32, tag="bc_sb", name="bc_sb")
                        nc.gpsimd.partition_broadcast(bc_sb[0:64, :], recip[0:1, :])
                        o_sb = att_pool.tile([64, 1024], f32, tag="o", name="o_sb")
                        nc.vector.tensor_mul(
                            o_sb[0:64, 0:512], avA[0:64, :], bc_sb[0:64, 0:512]
                        )
                        nc.vector.tensor_mul(
                            o_sb[0:64, 512:1024], avB[0:64, :], bc_sb[0:64, 512:1024]
                        )
                    nc.sync.dma_start(
                        outT[hA * 64:hA * 64 + 64, qsl], o_sb[0:64, 0:512]
                    )
                    nc.sync.dma_start(
                        outT[hB * 64:hB * 64 + 64, qsl], o_sb[0:64, 512:1024]
                    )
                    # interleave next pair's Q/K projection units into this tail
                    if p + 1 < NP:
                        emit_qk_unit("q", p + 1, qch)
                        emit_qk_unit("k", p + 1, qch)

    nc.compile()
    return nc


def _get_compiled():
    global _compiled
    if _compiled is None:
        _compiled = _build()
    return _compiled


def make_in_maps(q, k, v, Wq, bq, Wk, bk, Wv, bv):
    import ml_dtypes

    bf16 = ml_dtypes.bfloat16
    in_maps = []
    for c in range(_NCORES):
        b, g = c // 2, c % 2
        gsl = slice(g * _GSZ, (g + 1) * _GSZ)
        in_maps.append({
            "qT": np.ascontiguousarray(np.asarray(q)[b].T).astype(bf16),
            "kT": np.ascontiguousarray(np.asarray(k)[b].T).astype(bf16),
            "vT": np.ascontiguousarray(np.asarray(v)[b].T).astype(bf16),
            "WqT": np.ascontiguousarray(np.asarray(Wq)[gsl, :].T).astype(bf16),
            "WkT": np.ascontiguousarray(np.asarray(Wk)[gsl, :].T).astype(bf16),
            "WvT": np.ascontiguousarray(np.asarray(Wv)[gsl, :].T).astype(bf16),
            "bqT": np.ascontiguousarray(
                np.asarray(bq)[gsl].reshape(3, 128).T
            ).astype(np.float32),
            "bkT": np.ascontiguousarray(
                np.asarray(bk)[gsl].reshape(3, 128).T
            ).astype(np.float32),
            "bv_rep": np.tile(
                np.asarray(bv)[gsl][None, :], (128, 1)
            ).astype(np.float32),
        })
    return in_maps


def assemble_out(results):
    out = np.zeros((_BS, _SEQ, _DM), np.float32)
    for c in range(_NCORES):
        b, g = c // 2, c % 2
        out[b, :, g * _GSZ:(g + 1) * _GSZ] = np.asarray(
            results[c]["outT"], np.float32
        ).T
    return out


def kernel(q, k, v, Wq, bq, Wk, bk, Wv, bv):
    from concourse.bass_utils import run_bass_kernel_spmd

    nc = _get_compiled()
    in_maps = make_in_maps(q, k, v, Wq, bq, Wk, bk, Wv, bv)
    res = run_bass_kernel_spmd(nc, in_maps, core_ids=list(range(_NCORES)))
    return assemble_out(res.results)
